# revision 69
# baseline (speedup 1.0000x reference)
"""BagOfWords Trainium2 kernel (bf16 pipeline).

Reference computation (per batch b):
    emb    = emb_table[context]                      # (T, D) gather
    logits = emb @ W.T + b                           # (T, V)
    out[t] = (sum_{s<=t} (s+1) * logits[s]) / den[t] # weighted causal cum-avg
    den[t] = (t+1)(t+2)/2

Key identity: the weighted cumsum commutes with the GEMM:
    out[t, v] = (num[t] @ W[v]) / den[t] + b[v]
    num[t, d] = sum_{s<=t} (s+1) * emb[s, d]
so the O(T*V) cumsum collapses onto the tiny (T, D) embedding side.
On device, per 128-token chunk (PE / ACT):
    psum[d, t] = sum_s emb[s, d] * UTW_c[s, t]      # one matmul per d-chunk
    NT[d, t]   = psum[d, t] + NT_prev[d, last]      # ACT copy w/ bias
with UTW_c[s, t] = (c*128+s+1) * [s <= t] -- the position weights folded
into the per-chunk upper-triangular constant, so there is no separate
scale pass.  The carry between chunks is the previous chunk's LAST COLUMN
of NT, consumed as the per-partition bias of the ACT PSUM->SBUF copy.
NT comes out pre-transposed (d on partitions) = exactly the lhsT layout the
big GEMM wants.  Then out = (NT.T @ W.T) * (1/den[t]) with the normalization
applied as a per-partition scalar in the PSUM->SBUF eviction, split across
ACT and DVE (2 tiles each per 4-tile store group), and streamed to HBM.

Everything that touches HBM is bf16 (table, weights, output, consts);
PSUM accumulation stays fp32.  Measured end-to-end rel err ~3e-3 vs the
fp32 reference (gate is 2e-2).  bf16 halves DMA traffic (42 -> 21 MB/core)
and moves the bottleneck to the PE: the big GEMM is 192K moving columns
= 80us at 1 col/cycle/2.4GHz, which bf16 sustains (fp32 would be 4x).

A small fp32 shadow column (carry_sb) tracks each chunk's last NT column:
it feeds the next chunk's ACT bias (which must be fp32 on hardware) and
keeps the carry chain exact instead of re-rounding through bf16.

Sharding (8 cores): 4-way over B x 2-way over V.  Each core gathers 2
batches (2048 rows) but holds only half of W -- the DMA-optimal split.

Raw Bass with manual semaphores (one wait per instruction): the walrus build
in this container rejects instructions carrying multiple sem waits.

DMA semaphore discipline: a DMA's 16 per-SDMA-engine sem increments interleave
arbitrarily with other in-flight DMAs on the same semaphore, so every
concurrently-outstanding DMA group gets its own semaphore, waited to exactly
16 per iteration.

reps>1 repeats the whole pipeline inside one NEFF (used only for timing).
Iterations re-gather from the table so every rep computes identical values;
cross-iteration WAR hazards get explicit waits.
"""

import functools
import os
from contextlib import ExitStack

import numpy as np

import concourse.bass as bass
from concourse import mybir
from concourse.bass_utils import run_bass_kernel_spmd

B, T, V, D = 8, 1024, 8000, 384
P = 128
NCORE = 8
NCHUNK = T // P                 # 8 token chunks per batch
KD = D // P                     # 3 contraction chunks
NV = 500                        # vocab tile (one fp32 PSUM bank)
VGRP = 4                        # vocab tiles per store group
NSTAGE = 4                      # output staging buffers
GEMM_BANKS = 6
F32 = mybir.dt.float32
BF16 = mybir.dt.bfloat16

NVG = int(os.environ.get("BOW_NVG", "2"))   # vocab groups (1 or 2)
WARM = int(os.environ.get("BOW_WARM", "0"))   # PE ramp warm-up matmuls
NB = NVG                        # batches per core (B=8, 8 cores)
V_CORE = V // NVG               # vocab columns per core
BT = NB * T                     # tokens per core
NCHT = NB * NCHUNK              # token chunks per core
NTV = V_CORE // NV              # vocab tiles per core
NGRP = NTV // VGRP              # store column groups
GCOLS = VGRP * NV               # columns per weight/store group

# bf16 const-block column layout (single DMA, single sem)
C_UTW = 0                       # [128, 8*128] per-chunk (s+1)-weighted tril^T
C_DENROW = C_UTW + NCHUNK * P   # row 0, [1, 1024] den[t] (bias path only)
C_BIAS = C_DENROW + T           # row 0, [1, V_CORE] (only when has_bias)
CW16_NOBIAS = C_UTW + NCHUNK * P
CW16_BIAS = C_BIAS + V_CORE
# fp32 const block: idenc only
CW32 = NCHUNK                   # [128, 8] 1/den[c*128+p] column layout

# one single-chunk gather per 128 tokens: multi-chunk offset APs scramble the
# destination layout on real hardware (descgen ucode disagrees with the
# interpreter) and can write out of bounds, and dma_gather does not compile
# in this container's walrus -- so the SWDGE preps stay serial on Pool.
GATHER_GROUPS = [1] * NCHT
assert sum(GATHER_GROUPS) == NCHT
_GSTART = [sum(GATHER_GROUPS[:i]) for i in range(len(GATHER_GROUPS))]
_GROUP_OF = [max(i for i, s in enumerate(_GSTART) if s <= cc)
             for cc in range(NCHT)]

# per-iteration semaphore increments
CT_IT = NCHT * KD               # ctdone / ctsb
GM_IT = NCHT * NTV              # pegemm tiles
GR_IT = GM_IT // VGRP           # store groups
AE_IT = GM_IT // 2              # ACT evictions (tiles nin 0,1)
DE_IT = GM_IT // 2              # DVE evictions (tiles nin 2,3)


def _evict_count(a):
    """Engine-local (engine, 1-based count) for absolute gemm tile a.
    The very first block's four tiles all evict on DVE (ACT is busy with
    the startup carry/NT chain), shifting later counts by +-2."""
    if a < VGRP:
        return "d", a + 1
    r = a % VGRP
    if r < 2:
        return "a", 2 * (a // VGRP) + r - 1
    return "d", 2 * (a // VGRP) + r + 1


def _ctsb_count(it, nchunks):
    """DVE ctsb increments once `nchunks` chunks of iter `it` are copied."""
    return KD * (it * NCHT + nchunks)


def _build(has_bias: bool, reps: int = 1, dbg: bool = False):
    nc = bass.Bass("TRN2", target_bir_lowering=False, debug=False)

    CW16 = CW16_BIAS if has_bias else CW16_NOBIAS

    idx_d = nc.dram_tensor("idx", [P, NCHT], mybir.dt.int32, kind="ExternalInput")
    table_d = nc.dram_tensor("table", [V, D], BF16, kind="ExternalInput")
    wt_d = nc.dram_tensor("wt", [D, V_CORE], BF16, kind="ExternalInput")
    cst16_d = nc.dram_tensor("cst16", [P, CW16], BF16, kind="ExternalInput")
    cst32_d = nc.dram_tensor("cst32", [P, CW32], F32, kind="ExternalInput")
    out_d = nc.dram_tensor("out", [BT, V_CORE], BF16, kind="ExternalOutput")
    if dbg:
        demb_d = nc.dram_tensor("demb", [P, NCHT * D], BF16, kind="ExternalOutput")
        dct_d = [nc.dram_tensor(f"dct{k}", [P, BT], BF16, kind="ExternalOutput")
                 for k in range(KD)]
        dcarry_d = nc.dram_tensor("dcarry", [P, KD * NCHT], F32, kind="ExternalOutput")

    with ExitStack() as ctx:
        e = ctx.enter_context
        # SBUF
        idx_sb = e(nc.sbuf_tensor("idx_sb", [P, NCHT], mybir.dt.int32))
        cst16 = e(nc.sbuf_tensor("cst16_sb", [P, CW16], BF16))
        cst32 = e(nc.sbuf_tensor("cst32_sb", [P, CW32], F32))
        emb_sb = e(nc.sbuf_tensor("emb_sb", [P, NCHT * D], BF16))
        ct_sb = [e(nc.sbuf_tensor(f"ct{k}", [P, BT], BF16)) for k in range(KD)]
        # fp32 shadow of each chunk's last NT column: the ACT bias operand
        # must be fp32 on hardware, and fp32 carry also kills the bf16
        # carry-rounding accumulation
        carry_sb = e(nc.sbuf_tensor("carry_sb", [P, KD * NCHT], F32))
        wt_sb = [e(nc.sbuf_tensor(f"wt{k}", [P, V_CORE], BF16)) for k in range(KD)]
        ostg = [e(nc.sbuf_tensor(f"ostg{q}", [P, VGRP * NV], BF16)) for q in range(NSTAGE)]
        # PSUM (8 banks: 6 gemm + 2 prefix).  One prefix bank holds a whole
        # chunk (KD*128 = 384 fp32 cols), so the 3 per-chunk matmuls never
        # WAR against their own chunk's ACT copies -- only chunk cc-2's.
        gps = [e(nc.psum_tensor(f"gps{i}", [P, NV], F32)) for i in range(GEMM_BANKS)]
        ctps = [e(nc.psum_tensor(f"ctps{i}", [P, KD * P], F32)) for i in range(2)]
        # sems -- one per concurrently-outstanding DMA group
        csem16 = e(nc.semaphore("csem16"))
        csem32 = e(nc.semaphore("csem32"))
        wsem = [[e(nc.semaphore(f"wsem{k}_{g}")) for g in range(NGRP)] for k in range(KD)]
        wsemh = [[e(nc.semaphore(f"wsemh{k}_{g}")) for g in range(NGRP)] for k in range(KD)]
        # group-0 second-half weights split into per-tile 500-col pieces so
        # the first block's nin2/nin3 k-slices land as early as possible
        wq2 = [e(nc.semaphore(f"wq2_{k}")) for k in range(KD)]
        wq3 = [e(nc.semaphore(f"wq3_{k}")) for k in range(KD)]
        gidx = e(nc.semaphore("gidx"))
        gsem = [e(nc.semaphore(f"gsem{gg}")) for gg in range(len(GATHER_GROUPS))]
        osem = [e(nc.semaphore(f"osem{q}")) for q in range(NSTAGE)]
        # engine-progress sems (single-inc, exactly ordered)
        ctdone = e(nc.semaphore("ctdone"))  # prefix psum tiles done
        ctsb = e(nc.semaphore("ctsb"))      # NT psum->sbuf copies
        pegemm = e(nc.semaphore("pegemm"))  # gemm psum tiles done
        asem = e(nc.semaphore("asem"))      # ACT evictions
        dsem = e(nc.semaphore("dsem"))      # DVE evictions
        carrysem = e(nc.semaphore("carrysem"))  # ACT fp32 carry columns
        blk = e(nc.Block())

        def emb_cc(cc):
            return emb_sb[:, cc * D:(cc + 1) * D]

        utw_ap = lambda c: cst16[:, C_UTW + c * P:C_UTW + (c + 1) * P]
        idenc_ap = lambda c: cst32[:, c:c + 1]
        denrow_ap = lambda c: cst16[0:1, C_DENROW + c * P:C_DENROW + (c + 1) * P]
        bias_ap = lambda n: cst16[0:1, C_BIAS + n * NV:C_BIAS + (n + 1) * NV]

        @blk.sync
        def _(sync):
            # idx first: the gather chain (idx -> SWDGE prep -> gather 0) is
            # the critical path to the first prefix matmul
            sync.dma_start(idx_sb[:], idx_d[:]).then_inc(gidx, 16)
            sync.dma_start(cst16[:], cst16_d[:]).then_inc(csem16, 16)
            sync.dma_start(cst32[:], cst32_d[:]).then_inc(csem32, 16)
            # group-0 weights k-major in three bands (first half, then two
            # 500-col quarters) so each of the first block's tiles unblocks
            # as early as possible; group-1 halves held until the gathers
            # are through the DMA queue
            g0_bands = [(0, GCOLS // 2, wsemh), (GCOLS // 2, GCOLS // 2 + NV, wq2),
                        (GCOLS // 2 + NV, GCOLS, wq3)]
            for c0, c1, sems in g0_bands:
                for k in range(KD):
                    dma = sync.dma_start(wt_sb[k][:, c0:c1],
                                         wt_d[k * P:(k + 1) * P, c0:c1])
                    dma.then_inc(sems[k][0] if sems is wsemh else sems[k], 16)
            for h in range(2):
                for k in range(KD):
                    sync.wait_ge(gsem[min(h * 3 + k + 1,
                                          len(GATHER_GROUPS) - 1)], 16)
                    cols = slice(GCOLS + h * GCOLS // 2,
                                 GCOLS + (h + 1) * GCOLS // 2)
                    dma = sync.dma_start(wt_sb[k][:, cols],
                                         wt_d[k * P:(k + 1) * P, cols])
                    if h == 1:
                        dma.then_inc(wsem[k][1], 16)
                    else:
                        dma.then_inc(wsemh[k][1], 16)
            # output stores (SP's DGE ring is free once the weights are out)
            for it in range(reps):
                for g in range(NGRP):
                    for mc in range(NCHT):
                        gi = it * GR_IT + g * NCHT + mc
                        last_grp = gi == reps * GR_IT - 1
                        if not last_grp:
                            if gi > 0:
                                sync.wait_ge(asem, 2 * (gi + 1) - 2)
                            sync.wait_ge(dsem, 2 * (gi + 1) + (2 if gi > 0 else 2))
                            sync.dma_start(
                                out_d[mc * P:(mc + 1) * P,
                                      g * GCOLS:(g + 1) * GCOLS],
                                ostg[gi % NSTAGE][:],
                            ).then_inc(osem[gi % NSTAGE], 16)
                        else:
                            # final group: per-tile stores so each tile
                            # streams out as soon as its eviction lands
                            for nin in range(VGRP):
                                eng, cnt = _evict_count(gi * VGRP + nin)
                                sync.wait_ge(asem if eng == "a" else dsem, cnt)
                                sync.dma_start(
                                    out_d[mc * P:(mc + 1) * P,
                                          g * GCOLS + nin * NV:
                                          g * GCOLS + (nin + 1) * NV],
                                    ostg[gi % NSTAGE][:, nin * NV:(nin + 1) * NV],
                                ).then_inc(osem[gi % NSTAGE], 16)
            for q in range(NSTAGE):
                ngrp_q = (reps * GR_IT - q + NSTAGE - 1) // NSTAGE
                if (reps * GR_IT - 1) % NSTAGE == q:
                    ngrp_q += VGRP - 1   # final group incs osem per tile
                sync.wait_ge(osem[q], 16 * ngrp_q)
            if dbg:
                dbgsem = nc.semaphore("dbgsem").__enter__()
                sync.dma_start(demb_d[:], emb_sb[:]).then_inc(dbgsem, 16)
                for k in range(KD):
                    sync.dma_start(dct_d[k][:], ct_sb[k][:]).then_inc(dbgsem, 16)
                sync.dma_start(dcarry_d[:], carry_sb[:]).then_inc(dbgsem, 16)
                sync.wait_ge(dbgsem, 16 * (KD + 2))

        @blk.gpsimd
        def _(gpsimd):
            gpsimd.wait_ge(gidx, 16)
            for it in range(reps):
                for gg, ng in enumerate(GATHER_GROUPS):
                    c0 = _GSTART[gg]
                    if it > 0:
                        # WAR: PE must be done reading these chunks of iter it-1
                        gpsimd.wait_ge(ctdone, (it - 1) * CT_IT + (c0 + ng) * KD)
                    gpsimd.indirect_dma_start(
                        out=emb_sb[:, c0 * D:(c0 + ng) * D],
                        out_offset=None,
                        in_=table_d[:],
                        in_offset=bass.IndirectOffsetOnAxis(
                            ap=idx_sb[:, c0:c0 + ng], axis=0),
                    ).then_inc(gsem[gg], 16)

        def act_evict(scalar, a, mc):
            gi = a // VGRP
            nin = a % VGRP              # 0 or 1 on ACT
            if nin == 0 and gi >= NSTAGE:
                scalar.wait_ge(osem[gi % NSTAGE], 16 * (gi // NSTAGE))
            scalar.wait_ge(pegemm, a + 1)
            scalar.mul(ostg[gi % NSTAGE][:, nin * NV:(nin + 1) * NV],
                       gps[a % GEMM_BANKS][:],
                       idenc_ap(mc % NCHUNK)).then_inc(asem, 1)

        @blk.scalar
        def _(scalar):
            scalar.wait_ge(csem32, 16)
            for it in range(reps):
                def carr(cc):
                    # fp32 carry columns (psum's last column + prev carry),
                    # then the bf16 NT copy using that fp32 carry as bias.
                    jc = it * NCHT + cc
                    for k in range(KD):
                        j = it * CT_IT + cc * KD + k
                        scalar.wait_ge(ctdone, j + 1)
                        if it > 0 and cc == 0 and k == 0:
                            # WAR: gemm of iter it-1 must be done reading ct_sb
                            scalar.wait_ge(pegemm, it * GM_IT)
                        dst = carry_sb[:, cc * KD + k:cc * KD + k + 1]
                        src = ctps[jc % 2][:, k * P + P - 1:k * P + P]
                        ntdst = ct_sb[k][:, cc * P:(cc + 1) * P]
                        ntsrc = ctps[jc % 2][:, k * P:(k + 1) * P]
                        # NT first (the gemm waits on it), carry second
                        if cc % NCHUNK == 0:
                            scalar.copy(ntdst, ntsrc).then_inc(ctsb, 1)
                            scalar.copy(dst, src).then_inc(carrysem, 1)
                        else:
                            prev = carry_sb[:, (cc - 1) * KD + k:(cc - 1) * KD + k + 1]
                            scalar.add(ntdst, ntsrc, prev).then_inc(ctsb, 1)
                            scalar.add(dst, src, prev).then_inc(carrysem, 1)

                # carry columns lead the gemm by one chunk; ACT evictions for
                # the g0 sweep trail the block that produced them
                carr(0)
                for mc in range(NCHT):
                    if mc + 1 < NCHT:
                        carr(mc + 1)
                    a0 = it * GM_IT + mc * VGRP
                    if it == 0 and mc == 0:
                        continue   # first block's evictions run on DVE
                    act_evict(scalar, a0, mc)
                    act_evict(scalar, a0 + 1, mc)
                for g in range(1, NGRP):
                    for mc in range(NCHT):
                        a0 = it * GM_IT + (g * NCHT + mc) * VGRP
                        act_evict(scalar, a0, mc)
                        act_evict(scalar, a0 + 1, mc)

        @blk.tensor
        def _(tensor):
            # PE p-state warm-up: dummy matmuls on (uninitialized) SBUF while
            # the idx/gather DMA chain is in flight, so the real pipeline
            # starts at the full 2.4 GHz clock.  The garbage results land in a
            # ctps bank that the first real prefix overwrites (start=True),
            # and WAW on ctps is same-engine-ordered.
            for w in range(WARM):
                tensor.matmul(ctps[w % 2][:, 0:P], lhsT=emb_sb[:, 0:P],
                              rhs=emb_sb[:, 0:P], start=True, stop=True)
            tensor.wait_ge(csem16, 16)
            for it in range(reps):
                def prefix(cc):
                    # one matmul per (chunk, d-slice) into one bank; position
                    # weights are in UTW; the carry is applied by the ACT copy
                    tensor.wait_ge(gsem[_GROUP_OF[cc]], 16 * (it + 1))
                    jc = it * NCHT + cc
                    if jc >= 2:
                        # WAR on ctps bank: chunk jc-2's NT copies (DVE) and
                        # carry columns (ACT) must both be done
                        tensor.wait_ge(ctsb, KD * (jc - 1))
                        tensor.wait_ge(carrysem, KD * (jc - 1))
                    for k in range(KD):
                        tensor.matmul(
                            ctps[jc % 2][:, k * P:(k + 1) * P],
                            lhsT=emb_sb[:, cc * D + k * P: cc * D + (k + 1) * P],
                            rhs=utw_ap(cc % NCHUNK),
                            start=True, stop=True).then_inc(ctdone, 1)

                def gemm_block(g, mc, pf=None):
                    split0 = it == 0 and mc == 0   # chunk-0 NT copies split
                    if g == 0 and not split0:
                        tensor.wait_ge(ctsb, _ctsb_count(it, mc + 1))
                    for nin in range(VGRP):
                        n = g * VGRP + nin
                        a = it * GM_IT + (g * NCHT + mc) * VGRP + nin
                        if a >= GEMM_BANKS:
                            eng, cnt = _evict_count(a - GEMM_BANKS)
                            tensor.wait_ge(asem if eng == "a" else dsem, cnt)
                        for k in range(KD):
                            if split0 and g == 0 and nin == 0:
                                # per-k NT waits: start on each k-slice the
                                # moment its DVE copy lands
                                tensor.wait_ge(ctsb, k + 1)
                            if it == 0 and mc == 0 and nin == 0:
                                tensor.wait_ge(wsemh[k][g], 16)
                            if it == 0 and mc == 0 and nin == 2:
                                tensor.wait_ge(wq2[k] if g == 0 else wsem[k][g], 16)
                            if it == 0 and mc == 0 and nin == 3 and g == 0:
                                tensor.wait_ge(wq3[k], 16)
                            last = (k == KD - 1) and not has_bias
                            mm = tensor.matmul(
                                gps[a % GEMM_BANKS][:],
                                lhsT=ct_sb[k][:, mc * P:(mc + 1) * P],
                                rhs=wt_sb[k][:, n * NV:(n + 1) * NV],
                                start=(k == 0), stop=last)
                        if has_bias:
                            mm = tensor.matmul(
                                gps[a % GEMM_BANKS][:],
                                lhsT=denrow_ap(mc % NCHUNK),
                                rhs=bias_ap(n),
                                start=False, stop=True)
                        mm.then_inc(pegemm, 1)
                        if nin == 0 and pf is not None:
                            prefix(pf)

                # uniform sweep: prefix(mc+1) is interleaved after the first
                # tile of block (0, mc) so its NT copies overlap the block
                prefix(0)
                for mc in range(NCHT):
                    gemm_block(0, mc, pf=mc + 1 if mc + 1 < NCHT else None)
                for g in range(1, NGRP):
                    for mc in range(NCHT):
                        gemm_block(g, mc)

        @blk.vector
        def _(vector):
            vector.wait_ge(csem32, 16)

            def evict(it, g, mc, nin):
                a = it * GM_IT + (g * NCHT + mc) * VGRP + nin
                gi = a // VGRP
                if nin == 2 and gi >= NSTAGE:
                    vector.wait_ge(osem[gi % NSTAGE], 16 * (gi // NSTAGE))
                vector.wait_ge(pegemm, a + 1)
                vector.tensor_scalar_mul(
                    ostg[gi % NSTAGE][:, nin * NV:(nin + 1) * NV],
                    gps[a % GEMM_BANKS][:],
                    idenc_ap(mc % NCHUNK)).then_inc(dsem, 1)

            for it in range(reps):
                for g in range(NGRP):
                    for mc in range(NCHT):
                        if it == 0 and g == 0 and mc == 0:
                            evict(it, g, mc, 0)
                            evict(it, g, mc, 1)
                        evict(it, g, mc, 2)
                        evict(it, g, mc, 3)

    return nc


@functools.lru_cache(maxsize=None)
def _get_program(has_bias: bool, reps: int = 1, dbg: bool = False):
    return _build(has_bias, reps, dbg)


@functools.lru_cache(maxsize=None)
def _host_consts(has_bias: bool):
    import ml_dtypes
    CW16 = CW16_BIAS if has_bias else CW16_NOBIAS
    c16 = np.zeros((P, CW16), dtype=ml_dtypes.bfloat16)
    t = np.arange(T, dtype=np.float64)
    den = (t + 1.0) * (t + 2.0) / 2.0
    s = np.arange(P)
    tril_t = (s[:, None] <= s[None, :]).astype(np.float32)  # [s, t] s<=t
    for c in range(NCHUNK):
        posw = (np.arange(c * P, (c + 1) * P, dtype=np.float32) + 1.0)
        c16[:, C_UTW + c * P:C_UTW + (c + 1) * P] = (
            posw[:, None] * tril_t).astype(ml_dtypes.bfloat16)
    if has_bias:
        c16[0, C_DENROW:C_DENROW + T] = den.astype(ml_dtypes.bfloat16)
    c32 = np.zeros((P, CW32), dtype=np.float32)
    c32[:, :] = (1.0 / den).astype(np.float32).reshape(NCHUNK, P).T
    return c16, c32


def make_in_maps(context, emb_table, W, b):
    import ml_dtypes
    context = np.asarray(context)
    emb_table = np.asarray(emb_table, dtype=np.float32)
    W = np.asarray(W, dtype=np.float32)
    b = np.asarray(b, dtype=np.float32)
    has_bias = bool(np.any(b))

    table16 = np.ascontiguousarray(emb_table.astype(ml_dtypes.bfloat16))
    wt_full = np.ascontiguousarray(W.T.astype(ml_dtypes.bfloat16))  # (D, V)
    c16_0, c32 = _host_consts(has_bias)

    in_maps = []
    for ci in range(NCORE):
        vg, bg = ci % NVG, ci // NVG
        idx = np.concatenate(
            [context[bg * NB + bt].reshape(NCHUNK, P).T for bt in range(NB)],
            axis=1).astype(np.int32)           # [p, cc]
        wt = np.ascontiguousarray(wt_full[:, vg * V_CORE:(vg + 1) * V_CORE])
        c16 = c16_0
        if has_bias:
            c16 = c16_0.copy()
            c16[0, C_BIAS:C_BIAS + V_CORE] = \
                b[vg * V_CORE:(vg + 1) * V_CORE].astype(ml_dtypes.bfloat16)
        in_maps.append({"idx": np.ascontiguousarray(idx), "table": table16,
                        "wt": wt, "cst16": c16, "cst32": c32})
    return in_maps, has_bias


def kernel(context, emb_table, W, b):
    in_maps, has_bias = make_in_maps(context, emb_table, W, b)
    nc = _get_program(has_bias)
    res = None
    for attempt in range(3):
        try:
            res = run_bass_kernel_spmd(nc, in_maps, list(range(NCORE)))
            break
        except Exception:
            # the axon-tunneled device occasionally reports a transient
            # NRT_EXEC_UNIT_UNRECOVERABLE / INTERNAL error; back off and retry
            if attempt == 2:
                raise
            import time
            time.sleep(10.0 * (attempt + 1))
    out = np.empty((B, T, V), dtype=np.float32)
    for ci in range(NCORE):
        vg, bg = ci % NVG, ci // NVG
        o = np.asarray(res.results[ci]["out"]).astype(np.float32)
        for bt in range(NB):
            out[bg * NB + bt, :, vg * V_CORE:(vg + 1) * V_CORE] = \
                o[bt * T:(bt + 1) * T]
    return out
